# revision 29
# baseline (speedup 1.0000x reference)
"""LSTM encoder kernel for Trainium2 (Bass/Tile), data-parallel over batch on 8 cores.

Math (per core, batch shard B=256), sigmoid-only reparametrization:
  z = Wcat @ [h_{t-1} ; x_t]   with cc = c/2; g-gate columns of Wcat are
  scaled by 2, so a single sigmoid over all 128 gate rows yields
  S_g = sigmoid(2 z_g), i.e. tanh(z_g) = 2 S_g - 1.
  Gate row order [g, i, f, o]:
    S    = sigmoid(z + b)             (ACT, one op, 128 rows: Sg2@0, Si@32, Sf@64, So@96)
    t1   = S_g - 0.5                  (DVE, @0 -> @32; = tanh(z_g)/2; relocation is
                                       forced: STT/TT SBUF inputs must share a base partition)
    v    = S_f * cc_{t-1}             (Pool, @64/@64 -> @64; = f*c/2)
    u    = t1 * S_i                   (DVE, @32/@32 -> @64; = i*g/2)
    cc   = u + v                      (DVE, @64/@64 -> @64; = c_new/2)
    ts   = tanh(2*cc)                 (ACT, @64 -> @96; = tanh(c_new); tanh and
                                       sigmoid share one ACT table set)
    h    = ts * S_o                   (DVE TT, @96/@96 -> bf16 rhs slot @0)

Precision: weights/x/h in bf16 (PE 1 cycle/row, single LDWEIGHTS); S/v/u/cc/ts
in fp16 (enables DVE 2x perf modes). Weights are loaded into the PE array
ONCE via a standalone ldweights; per-step matmuls are emitted with
ldweights=False.

Scheduling: the per-step recurrence is a ~2.9us dependency chain whose cost is
dominated by per-instruction fixed latencies and semaphore-completion delays
(DVE self-sem fires ~220ns after an op ends; Pool's ~790ns). Two batch blocks
run staggered (~500ns, NOT half a step: the two ACT ops of a block are ~1.4us
apart, so a half-period offset would collide S1 with ts0). Every instruction
gets a tile_set_cur_wait tick at its steady-state start time from an offline
fixed-point model (order control for the list scheduler only). Same-engine
RAW pairs whose consumer is issue-distance >= 1 behind a TENSOR_SCALAR
producer, or >= 2 behind a TENSOR_TENSOR producer, are demoted to
ordering-only (nosync) deps to keep semaphore round-trips off the chain;
adjacent TT->TT RAW reads race on hardware and must keep their semaphore.
"""

import numpy as np
import ml_dtypes
from contextlib import ExitStack

import concourse.bass as bass
import concourse.tile as tile
from concourse import bacc, mybir
from concourse.bass_utils import run_bass_kernel_spmd

T_FULL = 512
B_FULL = 2048
IN = 10
H = 32
G = 4 * H          # 128 gate rows
K = IN + H         # 42 contraction rows of the combined matmul
NCORES = 8
B = B_FULL // NCORES  # 256 batch per core

NB = 2          # batch sub-blocks per core (latency pipelining)
FD = B // NB    # free-dim per block
TC = 16         # timesteps per SBUF chunk

# Per-step schedule (ns) from measured HW op timings; ticks steer the Tile
# list scheduler only (they fix per-engine queue order, not HW timing).
STEP_NS = 2550
# exact steady-state start times per (op, block) from the fixed-point model
SCHED = {
    0: dict(mm=0, S=378, t1=778, v=780, u=993, cc=1429, ts=1731, hh=2160),
    1: dict(mm=650, S=1028, t1=1428, v=1430, u=1643, cc=2079, ts=2381, hh=2810),
}

BF16 = mybir.dt.bfloat16
F16 = mybir.dt.float16
F32 = mybir.dt.float32
EW = F16            # elementwise dtype on the recurrence chain
SIG = mybir.ActivationFunctionType.Sigmoid
TANH = mybir.ActivationFunctionType.Tanh
MULT = mybir.AluOpType.mult
ADD = mybir.AluOpType.add
SUB = mybir.AluOpType.subtract

_CACHE = {}


def _mm_noldw(nc, out, lhsT, rhs):
    """MATMUL that reuses the PE-resident weights (no LDWEIGHTS emitted)."""
    te = nc.tensor
    ifmap_ap = te.lower_ap(rhs.opt({0}), opt=False)
    weights_ap = te.lower_ap(lhsT.opt({0}), opt=False, for_matmul_weights=True)
    out_ap = te.lower_ap(out)
    return te.add_instruction(
        mybir.InstMatmult(
            name=te.bass.get_next_instruction_name(),
            replication_resolution=0,
            replication_shift_amnt=0,
            replication_num_rows=0,
            start_tensor_calc=True,
            stop_tensor_calc=True,
            ins=[ifmap_ap, weights_ap],
            outs=[out_ap],
            perf_mode=None,
            is_transpose=None,
            ifmap_quant_offset=None,
            weights_quant_offset=None,
            bass_skip_group_check=False,
            tile_position=None,
            tile_size=None,
            ldweights=False,
        )
    )


def _build(t_total=T_FULL, tc=TC, nb=NB):
    fd = B // nb
    nchunk = t_total // tc
    nc = bacc.Bacc(trn_type="TRN2", debug=False, target_bir_lowering=False)

    xT = nc.dram_tensor("xT", [t_total, IN, B], BF16, kind="ExternalInput").ap()
    wcat = nc.dram_tensor("wcat", [K, G], BF16, kind="ExternalInput").ap()
    bg = nc.dram_tensor("bg", [G, 1], F32, kind="ExternalInput").ap()
    hout = nc.dram_tensor("hout", [t_total, H, B], BF16, kind="ExternalOutput").ap()

    with tile.TileContext(nc) as tc_, ExitStack() as ctx:
        const = ctx.enter_context(tc_.tile_pool(name="const", bufs=1))
        xpool = ctx.enter_context(tc_.tile_pool(name="xpool", bufs=3))
        spool = ctx.enter_context(tc_.tile_pool(name="spool", bufs=4))
        taupool = ctx.enter_context(tc_.tile_pool(name="taupool", bufs=4))
        cpool = ctx.enter_context(tc_.tile_pool(name="cpool", bufs=4))
        t1pool = ctx.enter_context(tc_.tile_pool(name="t1pool", bufs=4))
        vpool = ctx.enter_context(tc_.tile_pool(name="vpool", bufs=4))
        upool = ctx.enter_context(tc_.tile_pool(name="upool", bufs=4))
        pspool = ctx.enter_context(tc_.tile_pool(name="pspool", bufs=4, space="PSUM"))

        w_t = const.tile([K, G], BF16)
        nc.sync.dma_start(w_t[:], wcat)
        bg_t = const.tile([G, 1], F32)
        nc.sync.dma_start(bg_t[:], bg)

        # one-time weight load; every step's matmul reuses the resident array
        nc.tensor.ldweights(w_t[:])

        # rhs chunk tiles: [K, tc*B] bf16; rows 0:H = hh slots, rows H:K = x slots
        chunk_tiles = {}

        def get_chunk(ch):
            if ch not in chunk_tiles:
                t = xpool.tile([K, tc * B], BF16, name="rhs", tag="rhs")
                if ch < nchunk:
                    nc.sync.dma_start(
                        t[H:K].rearrange("p (t b) -> p t b", t=tc),
                        xT[ch * tc:(ch + 1) * tc].rearrange("t p b -> p t b"),
                    )
                chunk_tiles[ch] = t
            return chunk_tiles[ch]

        cur = get_chunk(0)
        # hh_{-1} = 0
        nc.vector.memset(cur[0:H, 0:B], 0.0)

        c_prev = []
        for blk in range(nb):
            c0 = cpool.tile([3 * H, fd], EW, name=f"cc{blk}", tag=f"cc{blk}")
            nc.vector.memset(c0[2 * H:3 * H], 0.0)
            c_prev.append(c0)

        def tick(ns):
            tc_.tile_set_cur_wait(ns / 1e6)

        def desync(cons, prod):
            """Demote a same-engine RAW sync dep to ordering-only (no
            semaphore wait). The in-order engine plus the element-streamed
            SBUF write pipeline provides the write->read interlock; this
            removes the ~220ns semaphore round-trip from the chain."""
            ci, pi = cons.ins, prod.ins
            if ci.try_remove_dependency(pi.name):
                ci.add_nosync_dependencies_from(
                    bass._bass_rust.InstructionNameOrderedSet([pi.name]))

        def step(blk, s):
            base = s * STEP_NS
            sch = SCHED[blk]
            ch_, s_ = divmod(s, tc)
            col = s_ * B + blk * fd
            rhs = get_chunk(ch_)
            tick(base + sch['mm'])
            p = pspool.tile([G, fd], F32, name="gates", tag=f"gates{blk}")
            _mm_noldw(nc, p[:], w_t[:], rhs[:, col:col + fd])
            # S = sigmoid(z): Sg2@0, Si@32, Sf@64, So@96
            tick(base + sch['S'])
            s_t = spool.tile([G, fd], EW, name="sgm", tag=f"sgm{blk}")
            i_S = nc.scalar.activation(s_t[:], p[:], SIG, bias=bg_t[:])
            # t1 = S_g - 0.5 relocated to @32 (pairs with S_i)
            tick(base + sch['t1'])
            t1 = t1pool.tile([2 * H, fd], EW, name="t1", tag=f"t1{blk}")
            i_t1 = nc.vector.tensor_scalar(t1[H:2 * H], s_t[0:H], 0.5, None, SUB)
            # v = f * cc_prev on Pool (keeps the DVE queue short)
            tick(base + sch['v'])
            v = vpool.tile([3 * H, fd], EW, name="v", tag=f"v{blk}")
            i_v = nc.gpsimd.tensor_tensor(
                v[2 * H:3 * H], s_t[2 * H:3 * H], c_prev[blk][2 * H:3 * H], MULT)
            # u = t1 * S_i  (= i*g/2)
            tick(base + sch['u'])
            u = upool.tile([3 * H, fd], EW, name="u", tag=f"u{blk}")
            i_u = nc.vector.tensor_tensor(
                u[2 * H:3 * H], t1[H:2 * H], s_t[H:2 * H], MULT)
            desync(i_u, i_t1)
            desync(i_u, i_S)
            tick(base + sch['cc'])
            c_new = cpool.tile([3 * H, fd], EW, name=f"ccn{blk}", tag=f"cc{blk}")
            i_cc = nc.vector.tensor_tensor(c_new[2 * H:3 * H], u[2 * H:3 * H],
                                           v[2 * H:3 * H], ADD)
            desync(i_cc, i_v)
            c_prev[blk] = c_new
            # ts = tanh(2*cc) = tanh(c) relocated to start 96 (pairs with o)
            tick(base + sch['ts'])
            ts = taupool.tile([G, fd], EW, name="ts", tag=f"ts{blk}")
            nc.scalar.activation(ts[3 * H:4 * H], c_new[2 * H:3 * H],
                                 TANH, scale=2.0)
            # h = ts * S_o -> bf16 rhs slot of step s+1
            tick(base + sch['hh'])
            ch2, s2 = divmod(s + 1, tc)
            col2 = s2 * B + blk * fd
            hdst = get_chunk(ch2)[0:H, col2:col2 + fd]
            nc.vector.tensor_tensor(
                hdst, ts[3 * H:4 * H], s_t[3 * H:4 * H], MULT)

        def emit_out(ch):
            cur_, nxt_ = get_chunk(ch), get_chunk(ch + 1)
            nc.sync.dma_start(
                hout[ch * tc:ch * tc + tc - 1].rearrange("t p b -> p t b"),
                cur_[0:H, B:].rearrange("p (t b) -> p t b", t=tc - 1),
            )
            nc.sync.dma_start(hout[ch * tc + tc - 1], nxt_[0:H, 0:B])

        for s in range(t_total):
            step(0, s)
            step(1, s)
            if s % tc == tc - 1:
                emit_out(s // tc)
    nc.compile()
    return nc


def _prep_weights(W_emb, b_emb, W_ih, W_hh, b_ih, b_hh):
    f8 = lambda a: np.asarray(a, np.float64)
    Wx = f8(W_ih) @ f8(W_emb)                                  # [4H, IN]
    bgv = f8(W_ih) @ f8(b_emb) + f8(b_ih) + f8(b_hh)           # [4H]
    perm = np.r_[2 * H:3 * H, 0:H, H:2 * H, 3 * H:4 * H]       # [g,i,f,o]
    wc = np.concatenate([f8(W_hh)[perm].T, Wx[perm].T], axis=0)  # [K, G]
    wc[:, 0:H] *= 2.0           # g-gate columns: sigmoid(2 z_g)
    bgv = bgv[perm].copy()
    bgv[0:H] *= 2.0
    return (np.ascontiguousarray(wc.astype(ml_dtypes.bfloat16)),
            np.ascontiguousarray(bgv.astype(np.float32).reshape(G, 1)))


def _run(x, W_emb, b_emb, W_ih, W_hh, b_ih, b_hh, trace=False):
    t_total = x.shape[0]
    key = (t_total, TC, NB)
    if key not in _CACHE:
        _CACHE[key] = _build(t_total, TC, NB)
    nc = _CACHE[key]

    wc, bgv = _prep_weights(W_emb, b_emb, W_ih, W_hh, b_ih, b_hh)
    x = np.asarray(x, np.float32)
    in_maps = []
    for c in range(NCORES):
        xs = np.ascontiguousarray(
            x[:, c * B:(c + 1) * B, :].transpose(0, 2, 1)).astype(
                ml_dtypes.bfloat16)  # [T, IN, B] bf16
        in_maps.append({"xT": xs, "wcat": wc, "bg": bgv})

    res = run_bass_kernel_spmd(nc, in_maps, list(range(NCORES)), trace=trace)
    out = np.empty((t_total, B_FULL, H), np.float32)
    for c in range(NCORES):
        out[:, c * B:(c + 1) * B, :] = np.asarray(
            res.results[c]["hout"], np.float32).transpose(0, 2, 1)
    return out, res


def kernel(x, W_emb, b_emb, W_ih, W_hh, b_ih, b_hh):
    out, _ = _run(x, W_emb, b_emb, W_ih, W_hh, b_ih, b_hh, trace=False)
    return out


# revision 30
# speedup vs baseline: 1.0344x; 1.0344x over previous
"""LSTM encoder kernel for Trainium2 (Bass/Tile), data-parallel over batch on 8 cores.

Math (per core, batch shard B=256), sigmoid-only reparametrization:
  z = Wcat @ [h_{t-1} ; x_t]   with cc = c/2; g-gate columns of Wcat are
  scaled by 2, so a single sigmoid over all 128 gate rows yields
  S_g = sigmoid(2 z_g), i.e. tanh(z_g) = 2 S_g - 1.
  Gate row order [g, i, f, o]:
    S    = sigmoid(z + b)             (ACT, one op, 128 rows: Sg2@0, Si@32, Sf@64, So@96)
    t1   = S_g - 0.5                  (DVE, @0 -> @32; = tanh(z_g)/2; relocation is
                                       forced: STT/TT SBUF inputs must share a base partition)
    v    = S_f * cc_{t-1}             (Pool, @64/@64 -> @64; = f*c/2)
    u    = t1 * S_i                   (DVE, @32/@32 -> @64; = i*g/2)
    cc   = u + v                      (DVE, @64/@64 -> @64; = c_new/2)
    ts   = tanh(2*cc)                 (ACT, @64 -> @96; = tanh(c_new); tanh and
                                       sigmoid share one ACT table set)
    h    = ts * S_o                   (DVE TT, @96/@96 -> bf16 rhs slot @0)

Precision: weights/x/h in bf16 (PE 1 cycle/row, single LDWEIGHTS); S/v/u/cc/ts
in fp16 (enables DVE 2x perf modes). Weights are loaded into the PE array
ONCE via a standalone ldweights; per-step matmuls are emitted with
ldweights=False.

Scheduling: the per-step recurrence is a ~2.9us dependency chain whose cost is
dominated by per-instruction fixed latencies and semaphore-completion delays
(DVE self-sem fires ~220ns after an op ends; Pool's ~790ns). Two batch blocks
run staggered (~500ns, NOT half a step: the two ACT ops of a block are ~1.4us
apart, so a half-period offset would collide S1 with ts0). Every instruction
gets a tile_set_cur_wait tick at its steady-state start time from an offline
fixed-point model (order control for the list scheduler only). Same-engine
RAW pairs whose consumer is issue-distance >= 1 behind a TENSOR_SCALAR
producer, or >= 2 behind a TENSOR_TENSOR producer, are demoted to
ordering-only (nosync) deps to keep semaphore round-trips off the chain;
adjacent TT->TT RAW reads race on hardware and must keep their semaphore.
"""

import numpy as np
import ml_dtypes
from contextlib import ExitStack

import concourse.bass as bass
import concourse.tile as tile
from concourse import bacc, mybir
from concourse.bass_utils import run_bass_kernel_spmd

T_FULL = 512
B_FULL = 2048
IN = 10
H = 32
G = 4 * H          # 128 gate rows
K = IN + H         # 42 contraction rows of the combined matmul
NCORES = 8
B = B_FULL // NCORES  # 256 batch per core

NB = 2          # batch sub-blocks per core (latency pipelining)
FD = B // NB    # free-dim per block
TC = 16         # timesteps per SBUF chunk

# Per-step schedule (ns) from measured HW op timings; ticks steer the Tile
# list scheduler only (they fix per-engine queue order, not HW timing).
STEP_NS = 2854
# exact steady-state start times per (op, block) from the fixed-point model
SCHED = {
    0: dict(mm=0, S=378, t1=777, v=902, u=1184, cc=1646, ts=1959, hh=2579),
    1: dict(mm=495, S=873, t1=1431, v=1397, u=1893, cc=2332, ts=2645, hh=3074),
}

BF16 = mybir.dt.bfloat16
F16 = mybir.dt.float16
F32 = mybir.dt.float32
EW = F16            # elementwise dtype on the recurrence chain
SIG = mybir.ActivationFunctionType.Sigmoid
TANH = mybir.ActivationFunctionType.Tanh
MULT = mybir.AluOpType.mult
ADD = mybir.AluOpType.add
SUB = mybir.AluOpType.subtract

_CACHE = {}


def _mm_noldw(nc, out, lhsT, rhs):
    """MATMUL that reuses the PE-resident weights (no LDWEIGHTS emitted)."""
    te = nc.tensor
    ifmap_ap = te.lower_ap(rhs.opt({0}), opt=False)
    weights_ap = te.lower_ap(lhsT.opt({0}), opt=False, for_matmul_weights=True)
    out_ap = te.lower_ap(out)
    return te.add_instruction(
        mybir.InstMatmult(
            name=te.bass.get_next_instruction_name(),
            replication_resolution=0,
            replication_shift_amnt=0,
            replication_num_rows=0,
            start_tensor_calc=True,
            stop_tensor_calc=True,
            ins=[ifmap_ap, weights_ap],
            outs=[out_ap],
            perf_mode=None,
            is_transpose=None,
            ifmap_quant_offset=None,
            weights_quant_offset=None,
            bass_skip_group_check=False,
            tile_position=None,
            tile_size=None,
            ldweights=False,
        )
    )


def _build(t_total=T_FULL, tc=TC, nb=NB):
    fd = B // nb
    nchunk = t_total // tc
    nc = bacc.Bacc(trn_type="TRN2", debug=False, target_bir_lowering=False)

    xT = nc.dram_tensor("xT", [t_total, IN, B], BF16, kind="ExternalInput").ap()
    wcat = nc.dram_tensor("wcat", [K, G], BF16, kind="ExternalInput").ap()
    bg = nc.dram_tensor("bg", [G, 1], F32, kind="ExternalInput").ap()
    hout = nc.dram_tensor("hout", [t_total, H, B], BF16, kind="ExternalOutput").ap()

    with tile.TileContext(nc) as tc_, ExitStack() as ctx:
        const = ctx.enter_context(tc_.tile_pool(name="const", bufs=1))
        xpool = ctx.enter_context(tc_.tile_pool(name="xpool", bufs=3))
        spool = ctx.enter_context(tc_.tile_pool(name="spool", bufs=4))
        taupool = ctx.enter_context(tc_.tile_pool(name="taupool", bufs=4))
        cpool = ctx.enter_context(tc_.tile_pool(name="cpool", bufs=4))
        t1pool = ctx.enter_context(tc_.tile_pool(name="t1pool", bufs=4))
        vpool = ctx.enter_context(tc_.tile_pool(name="vpool", bufs=4))
        upool = ctx.enter_context(tc_.tile_pool(name="upool", bufs=4))
        pspool = ctx.enter_context(tc_.tile_pool(name="pspool", bufs=4, space="PSUM"))

        w_t = const.tile([K, G], BF16)
        nc.sync.dma_start(w_t[:], wcat)
        bg_t = const.tile([G, 1], F32)
        nc.sync.dma_start(bg_t[:], bg)

        # one-time weight load; every step's matmul reuses the resident array
        nc.tensor.ldweights(w_t[:])

        # rhs chunk tiles: [K, tc*B] bf16; rows 0:H = hh slots, rows H:K = x slots
        chunk_tiles = {}

        def get_chunk(ch):
            if ch not in chunk_tiles:
                t = xpool.tile([K, tc * B], BF16, name="rhs", tag="rhs")
                if ch < nchunk:
                    nc.sync.dma_start(
                        t[H:K].rearrange("p (t b) -> p t b", t=tc),
                        xT[ch * tc:(ch + 1) * tc].rearrange("t p b -> p t b"),
                    )
                chunk_tiles[ch] = t
            return chunk_tiles[ch]

        cur = get_chunk(0)
        # hh_{-1} = 0
        nc.vector.memset(cur[0:H, 0:B], 0.0)

        c_prev = []
        for blk in range(nb):
            c0 = cpool.tile([3 * H, fd], EW, name=f"cc{blk}", tag=f"cc{blk}")
            nc.vector.memset(c0[2 * H:3 * H], 0.0)
            c_prev.append(c0)

        def tick(ns):
            tc_.tile_set_cur_wait(ns / 1e6)

        def desync(cons, prod):
            """Demote a same-engine RAW sync dep to ordering-only (no
            semaphore wait). The in-order engine plus the element-streamed
            SBUF write pipeline provides the write->read interlock; this
            removes the ~220ns semaphore round-trip from the chain."""
            ci, pi = cons.ins, prod.ins
            if ci.try_remove_dependency(pi.name):
                ci.add_nosync_dependencies_from(
                    bass._bass_rust.InstructionNameOrderedSet([pi.name]))

        def step(blk, s):
            base = s * STEP_NS
            sch = SCHED[blk]
            ch_, s_ = divmod(s, tc)
            col = s_ * B + blk * fd
            rhs = get_chunk(ch_)
            tick(base + sch['mm'])
            p = pspool.tile([G, fd], F32, name="gates", tag=f"gates{blk}")
            _mm_noldw(nc, p[:], w_t[:], rhs[:, col:col + fd])
            # S = sigmoid(z): Sg2@0, Si@32, Sf@64, So@96
            tick(base + sch['S'])
            s_t = spool.tile([G, fd], EW, name="sgm", tag=f"sgm{blk}")
            i_S = nc.scalar.activation(s_t[:], p[:], SIG, bias=bg_t[:])
            # t1 = S_g - 0.5 relocated to @32 (pairs with S_i)
            tick(base + sch['t1'])
            t1 = t1pool.tile([2 * H, fd], EW, name="t1", tag=f"t1{blk}")
            i_t1 = nc.vector.tensor_scalar(t1[H:2 * H], s_t[0:H], 0.5, None, SUB)
            # v = f * cc_prev on Pool (keeps the DVE queue short)
            tick(base + sch['v'])
            v = vpool.tile([3 * H, fd], EW, name="v", tag=f"v{blk}")
            i_v = nc.gpsimd.tensor_tensor(
                v[2 * H:3 * H], s_t[2 * H:3 * H], c_prev[blk][2 * H:3 * H], MULT)
            # u = t1 * S_i  (= i*g/2)
            tick(base + sch['u'])
            u = upool.tile([3 * H, fd], EW, name="u", tag=f"u{blk}")
            i_u = nc.vector.tensor_tensor(
                u[2 * H:3 * H], t1[H:2 * H], s_t[H:2 * H], MULT)
            desync(i_u, i_t1)
            desync(i_u, i_S)
            tick(base + sch['cc'])
            c_new = cpool.tile([3 * H, fd], EW, name=f"ccn{blk}", tag=f"cc{blk}")
            i_cc = nc.vector.tensor_tensor(c_new[2 * H:3 * H], u[2 * H:3 * H],
                                           v[2 * H:3 * H], ADD)
            desync(i_cc, i_u)
            c_prev[blk] = c_new
            # ts = tanh(2*cc) = tanh(c) relocated to start 96 (pairs with o)
            tick(base + sch['ts'])
            ts = taupool.tile([G, fd], EW, name="ts", tag=f"ts{blk}")
            nc.scalar.activation(ts[3 * H:4 * H], c_new[2 * H:3 * H],
                                 TANH, scale=2.0)
            # h = ts * S_o -> bf16 rhs slot of step s+1
            tick(base + sch['hh'])
            ch2, s2 = divmod(s + 1, tc)
            col2 = s2 * B + blk * fd
            hdst = get_chunk(ch2)[0:H, col2:col2 + fd]
            nc.vector.tensor_tensor(
                hdst, ts[3 * H:4 * H], s_t[3 * H:4 * H], MULT)

        def emit_out(ch):
            cur_, nxt_ = get_chunk(ch), get_chunk(ch + 1)
            nc.sync.dma_start(
                hout[ch * tc:ch * tc + tc - 1].rearrange("t p b -> p t b"),
                cur_[0:H, B:].rearrange("p (t b) -> p t b", t=tc - 1),
            )
            nc.sync.dma_start(hout[ch * tc + tc - 1], nxt_[0:H, 0:B])

        for s in range(t_total):
            step(0, s)
            step(1, s)
            if s % tc == tc - 1:
                emit_out(s // tc)
    nc.compile()
    return nc


def _prep_weights(W_emb, b_emb, W_ih, W_hh, b_ih, b_hh):
    f8 = lambda a: np.asarray(a, np.float64)
    Wx = f8(W_ih) @ f8(W_emb)                                  # [4H, IN]
    bgv = f8(W_ih) @ f8(b_emb) + f8(b_ih) + f8(b_hh)           # [4H]
    perm = np.r_[2 * H:3 * H, 0:H, H:2 * H, 3 * H:4 * H]       # [g,i,f,o]
    wc = np.concatenate([f8(W_hh)[perm].T, Wx[perm].T], axis=0)  # [K, G]
    wc[:, 0:H] *= 2.0           # g-gate columns: sigmoid(2 z_g)
    bgv = bgv[perm].copy()
    bgv[0:H] *= 2.0
    return (np.ascontiguousarray(wc.astype(ml_dtypes.bfloat16)),
            np.ascontiguousarray(bgv.astype(np.float32).reshape(G, 1)))


def _run(x, W_emb, b_emb, W_ih, W_hh, b_ih, b_hh, trace=False):
    t_total = x.shape[0]
    key = (t_total, TC, NB)
    if key not in _CACHE:
        _CACHE[key] = _build(t_total, TC, NB)
    nc = _CACHE[key]

    wc, bgv = _prep_weights(W_emb, b_emb, W_ih, W_hh, b_ih, b_hh)
    x = np.asarray(x, np.float32)
    in_maps = []
    for c in range(NCORES):
        xs = np.ascontiguousarray(
            x[:, c * B:(c + 1) * B, :].transpose(0, 2, 1)).astype(
                ml_dtypes.bfloat16)  # [T, IN, B] bf16
        in_maps.append({"xT": xs, "wcat": wc, "bg": bgv})

    res = run_bass_kernel_spmd(nc, in_maps, list(range(NCORES)), trace=trace)
    out = np.empty((t_total, B_FULL, H), np.float32)
    for c in range(NCORES):
        out[:, c * B:(c + 1) * B, :] = np.asarray(
            res.results[c]["hout"], np.float32).transpose(0, 2, 1)
    return out, res


def kernel(x, W_emb, b_emb, W_ih, W_hh, b_ih, b_hh):
    out, _ = _run(x, W_emb, b_emb, W_ih, W_hh, b_ih, b_hh, trace=False)
    return out
